# revision 1
# baseline (speedup 1.0000x reference)
"""Distributed kNN OOD-score kernel for 8 Trainium2 NeuronCores.

Problem: for each of 4*32*32 query vectors (D=768), find the 3 nearest
database vectors (N=20000, squared-L2), average the 3 distances, and
bilinearly upsample the resulting [4,32,32] map to [4,1,512,512].

Sharding: queries are data-parallel. Each core owns half of one batch
image (16 of 32 query rows = 512 queries); the database is replicated
and streamed through SBUF in bf16. The one halo row each core needs for
the 16x bilinear upsample is exchanged with its pair core via a tiny
AllGather. Each core computes the 4-row block its PAIR needs first
(local tile 0), so the AllGather launches ~40us before the matmul
stream ends and its ~15us latency is fully hidden. The per-core
interpolation matrix (host input) absorbs the resulting row permutation,
keeping the device program SPMD-uniform.

Per-core device program:
  - scores t[q,n] = q.x - ||x||^2/2 via TensorE: 6 bf16 K=128 matmuls
    (stationary query tile, moving db columns) + one K=2 matmul adding
    the -||x||^2/2 row in split-bf16 (hi+lo) precision, accumulated f32
    in PSUM.
  - ScalarE evacuates each 500-col PSUM bank into an SBUF score strip.
  - VectorE max8 per [128,4000] strip -> per-strip top-8; final max8
    over the 40 strip winners -> global top-3 per query (values only).
  - mean distance = reduce_sum of sqrt((q^2 - 2t)/9) (ScalarE fused
    scale+bias+sqrt).
  - pair AllGather of local tile 0's 128 ood values (boundary block).
  - 16x bilinear upsample = two small f32 matmuls with interpolation
    matrices (built on host; verified against jax.image.resize).
"""

import sys

if "/opt/trn_rl_repo" not in sys.path:
    sys.path.insert(0, "/opt/trn_rl_repo")

import numpy as np
import ml_dtypes

import concourse.bass as bass
import concourse.bacc as bacc
import concourse.mybir as mybir
import concourse.tile as tile
from concourse import bass_utils

# Problem shape (hardcoded per contract).
B, D, H, W = 4, 768, 32, 32
N = 20000
K_NN = 3
OUT_H = OUT_W = 512
N_CORES = 8

SC = 4000           # db columns per super-chunk (8 PSUM banks * 500)
N_SC = N // SC      # 5
BANK = 500
N_BANK = SC // BANK  # 8
HALF = SC // 2      # db DMA granularity (finer for startup overlap)
KC = D // 128       # 6 contraction chunks
QPC = 512           # queries scored per core (16 rows)
N_QT = QPC // 128   # 4
OROWS = 256         # output rows per core
NCOL = 24           # ood columns entering the upsample (16 own + 2x4 gathered)

F32 = mybir.dt.float32
BF16 = mybir.dt.bfloat16
AX = mybir.AxisListType
AF = mybir.ActivationFunctionType

# local tile -> 4-row block of this core's half (block i = rows 4i..4i+3).
# Tile 0 is the block the PAIR core needs as its halo row: for the top
# half (rows 0-15) that's block 3 (row 15), for the bottom half (rows
# 16-31) block 0 (row 16).
TILE_BLOCKS = ([3, 0, 1, 2], [0, 1, 2, 3])


def _build_program():
    nc = bacc.Bacc(
        "TRN2", target_bir_lowering=False, debug=False, num_devices=N_CORES
    )
    dbT = nc.dram_tensor("dbT", [D, N], BF16, kind="ExternalInput").ap()
    xh = nc.dram_tensor("xh", [2, N], BF16, kind="ExternalInput").ap()
    qT = nc.dram_tensor("qT", [128, KC * QPC], BF16, kind="ExternalInput").ap()
    q2 = nc.dram_tensor("q2", [128, N_QT], F32, kind="ExternalInput").ap()
    art = nc.dram_tensor("art", [NCOL, OROWS], F32, kind="ExternalInput").ap()
    ac = nc.dram_tensor("ac", [W, OUT_W], F32, kind="ExternalInput").ap()
    out = nc.dram_tensor("out", [OROWS, OUT_W], F32, kind="ExternalOutput").ap()

    with tile.TileContext(nc) as tc:
        with (
            tc.tile_pool(name="static", bufs=1) as sp,
            tc.tile_pool(name="db", bufs=4 * KC) as dbp,
            tc.tile_pool(name="scores", bufs=2) as scp,
            tc.tile_pool(name="small", bufs=4) as smp,
            tc.tile_pool(name="psum", bufs=N_BANK, space="PSUM") as pp,
            tc.tile_pool(name="dram", bufs=1, space="DRAM") as dp,
        ):
            # queries per k-chunk so the first matmul only waits for k=0;
            # first super-chunk's db tiles loaded before everything else
            qk_sb = [
                sp.tile([128, QPC], BF16, name=f"qk{k}") for k in range(KC)
            ]
            db0_tiles = []
            for k in range(KC):
                nc.sync.dma_start(qk_sb[k][:], qT[:, k * QPC : (k + 1) * QPC])
                for h in range(2):
                    t = dbp.tile([128, HALF], BF16, tag="db", name=f"db0_{k}_{h}")
                    nc.sync.dma_start(
                        t[:], dbT[k * 128 : (k + 1) * 128, h * HALF : (h + 1) * HALF]
                    )
                    db0_tiles.append(t)
            xh_sb = sp.tile([2, N], BF16)
            nc.sync.dma_start(xh_sb[:], xh[:])
            q2_sb = sp.tile([128, N_QT], F32)
            nc.sync.dma_start(q2_sb[:], q2[:])
            art_sb = sp.tile([NCOL, OROWS], F32)
            nc.sync.dma_start(art_sb[:], art[:])
            ac_sb = sp.tile([W, OUT_W], F32)
            nc.sync.dma_start(ac_sb[:], ac[:])
            ones2 = sp.tile([2, 128], BF16)
            nc.gpsimd.memset(ones2[:], 1.0)

            # per-query-tile top-8 winners of each (super-chunk, bank)
            parts = [
                sp.tile([128, N_SC * N_BANK * 8], F32, name=f"part{qt}")
                for qt in range(N_QT)
            ]
            # qt 0 separate so the collective only depends on it
            oods = [
                sp.tile([128, 1], F32, name=f"ood{qt}") for qt in range(N_QT)
            ]
            cc_in = dp.tile([128], F32)
            cc_out = dp.tile([256], F32)
            scratch = dp.tile([QPC], F32)

            for sc in range(N_SC):
                if sc == 0:
                    db_tiles = db0_tiles
                else:
                    db_tiles = []
                    for k in range(KC):
                        for h in range(2):
                            t = dbp.tile(
                                [128, HALF], BF16, tag="db", name=f"db{k}_{h}"
                            )
                            nc.sync.dma_start(
                                t[:],
                                dbT[
                                    k * 128 : (k + 1) * 128,
                                    sc * SC + h * HALF : sc * SC + (h + 1) * HALF,
                                ],
                            )
                            db_tiles.append(t)
                for qt in range(N_QT):
                    bank_sb = [
                        scp.tile([128, BANK], F32, tag=f"sb{b}", name=f"sb{b}")
                        for b in range(N_BANK)
                    ]
                    banks = [
                        pp.tile([128, BANK], F32, tag="bank", name=f"bank{b}")
                        for b in range(N_BANK)
                    ]
                    for k in range(KC):
                        lhsT = qk_sb[k][:, qt * 128 : (qt + 1) * 128]
                        for b in range(N_BANK):
                            src = db_tiles[2 * k + (b * BANK) // HALF]
                            off = (b * BANK) % HALF
                            nc.tensor.matmul(
                                banks[b][:],
                                lhsT,
                                src[:, off : off + BANK],
                                start=(k == 0),
                                stop=False,
                            )
                    for b in range(N_BANK):
                        nc.tensor.matmul(
                            banks[b][:],
                            ones2[:],
                            xh_sb[:, sc * SC + b * BANK : sc * SC + (b + 1) * BANK],
                            start=False,
                            stop=True,
                        )
                    for b in range(N_BANK):
                        nc.scalar.activation(bank_sb[b][:], banks[b][:], AF.Copy)
                        nc.vector.max(
                            parts[qt][
                                :, (sc * N_BANK + b) * 8 : (sc * N_BANK + b + 1) * 8
                            ],
                            bank_sb[b][:],
                        )

                    if sc != N_SC - 1:
                        continue
                    # epilogue inline after this qt's last strip (engines run
                    # their queues in order — emitting it later would trap it
                    # behind the remaining strip maxes)
                    f8 = smp.tile([128, 8], F32, tag="f8")
                    nc.vector.max(f8[:], parts[qt][:])
                    # dist_j/3 = sqrt((q2 - 2 t_j) / 9); host passes q2/9
                    d3 = smp.tile([128, K_NN], F32, tag="d3")
                    nc.scalar.activation(
                        d3[:],
                        f8[:, 0:K_NN],
                        AF.Sqrt,
                        bias=q2_sb[:, qt : qt + 1],
                        scale=-2.0 / 9.0,
                    )
                    nc.vector.reduce_sum(oods[qt][:], d3[:], axis=AX.X)
                    if qt == 0:
                        # boundary block: gather it across the pair ASAP so
                        # the ~15us collective hides under remaining matmuls
                        nc.sync.dma_start(cc_in[:], oods[0][:])
                        nc.gpsimd.collective_compute(
                            "AllGather",
                            mybir.AluOpType.bypass,
                            replica_groups=[[0, 1], [2, 3], [4, 5], [6, 7]],
                            ins=[cc_in.opt()],
                            outs=[cc_out.opt()],
                        )
                    # own ood values -> scratch incrementally (local order)
                    nc.sync.dma_start(
                        scratch.rearrange("(q p) -> p q", p=128)[:, qt : qt + 1],
                        oods[qt][:],
                    )

            # ood_hT[c, j]: j 0..15 own rows (local order), 16..23 the two
            # gathered boundary blocks in rank order
            ood_hT = sp.tile([W, NCOL], F32)
            nc.sync.dma_start(
                ood_hT[:, 0:16], scratch.rearrange("(r c) -> c r", c=W)
            )
            nc.sync.dma_start(
                ood_hT[:, 16:NCOL],
                cc_out.rearrange("(b r c) -> c (b r)", b=2, c=W),
            )

            # P1[j, ow] = sum_c ood_hT[c, j] * A_c[c, ow]
            p1 = pp.tile([NCOL, OUT_W], F32, tag="bank")
            nc.tensor.matmul(p1[:], ood_hT[:], ac_sb[:], start=True, stop=True)
            p1_sb = sp.tile([NCOL, OUT_W], F32)
            nc.scalar.activation(p1_sb[:], p1[:], AF.Copy)
            # out[oi, ow] = sum_j art[j, oi] * P1[j, ow]
            for m in range(2):
                p2 = pp.tile([128, OUT_W], F32, tag="bank", name=f"p2_{m}")
                nc.tensor.matmul(
                    p2[:],
                    art_sb[:, m * 128 : (m + 1) * 128],
                    p1_sb[:],
                    start=True,
                    stop=True,
                )
                o_sb = smp.tile([128, OUT_W], F32, tag="osb", name=f"osb{m}")
                nc.scalar.activation(o_sb[:], p2[:], AF.Copy)
                nc.sync.dma_start(out[m * 128 : (m + 1) * 128, :], o_sb[:])

    nc.compile()
    return nc


def _bilinear_matrix(out_size: int, in_size: int) -> np.ndarray:
    """Half-pixel (align_corners=False) bilinear interpolation matrix
    [out_size, in_size]; edge-clamped, equivalent to jax.image.resize
    'bilinear' for integer upsampling."""
    A = np.zeros((out_size, in_size), dtype=np.float64)
    scale = in_size / out_size
    for i in range(out_size):
        s = (i + 0.5) * scale - 0.5
        j0 = int(np.floor(s))
        w = s - j0
        A[i, min(max(j0, 0), in_size - 1)] += 1.0 - w
        A[i, min(max(j0 + 1, 0), in_size - 1)] += w
    return A.astype(np.float32)


_NC_CACHE = None


def _get_nc():
    global _NC_CACHE
    if _NC_CACHE is None:
        _NC_CACHE = _build_program()
    return _NC_CACHE


def make_in_maps(embeddings: np.ndarray, database: np.ndarray):
    embeddings = np.asarray(embeddings, dtype=np.float32)
    database = np.asarray(database, dtype=np.float32)

    dbT = np.ascontiguousarray(database.T).astype(ml_dtypes.bfloat16)
    # -||x||^2/2 in split bf16 (hi + lo)
    xh_f = -0.5 * np.einsum("nd,nd->n", database, database)
    hi = xh_f.astype(ml_dtypes.bfloat16)
    lo = (xh_f - hi.astype(np.float32)).astype(ml_dtypes.bfloat16)
    xh = np.stack([hi, lo])

    q_all = embeddings.transpose(0, 2, 3, 1).reshape(B, H * W, D)
    Ac = _bilinear_matrix(OUT_W, W)                      # [512, 32]
    Ar = _bilinear_matrix(OUT_H, H)                      # [512, 32]
    # the two gathered blocks in cc_out rank order: pair-core tile 0 rows
    cc_rows = [12, 13, 14, 15, 16, 17, 18, 19]

    in_maps = []
    for c in range(N_CORES):
        b, half = divmod(c, 2)
        blocks = TILE_BLOCKS[half]
        own_rows = [16 * half + 4 * blk + r for blk in blocks for r in range(4)]

        # queries in local-tile order
        q = np.concatenate(
            [
                q_all[b, (16 * half + 4 * blk) * W : (16 * half + 4 * blk + 4) * W]
                for blk in blocks
            ]
        )                                                # [512, 768]
        qTb = (
            np.ascontiguousarray(q.T)                    # [768, 512]
            .reshape(KC, 128, QPC)
            .transpose(1, 0, 2)
            .reshape(128, KC * QPC)
            .astype(ml_dtypes.bfloat16)
        )
        q2 = np.einsum("qd,qd->q", q, q) / 9.0
        q2 = np.ascontiguousarray(q2.reshape(N_QT, 128).T.astype(np.float32))

        # interpolation rows matching ood_hT's column order
        Arh = Ar[half * OROWS : (half + 1) * OROWS]      # [256, 32]
        art = np.zeros((NCOL, OROWS), dtype=np.float32)
        for j, row in enumerate(own_rows):
            art[j] = Arh[:, row]
        for j, row in enumerate(cc_rows):
            if row not in own_rows:
                art[16 + j] = Arh[:, row]
        in_maps.append(
            {
                "dbT": dbT,
                "xh": xh,
                "qT": qTb,
                "q2": q2,
                "art": art,
                "ac": np.ascontiguousarray(Ac.T),        # [32, 512]
            }
        )
    return in_maps


def run_device(in_maps, **kwargs):
    nc = _get_nc()
    return bass_utils.run_bass_kernel_spmd(
        nc, in_maps, core_ids=list(range(N_CORES)), **kwargs
    )


def kernel(embeddings, database, k, out_h, out_w):
    assert int(k) == K_NN and int(out_h) == OUT_H and int(out_w) == OUT_W
    in_maps = make_in_maps(np.asarray(embeddings), np.asarray(database))
    res = run_device(in_maps)
    out = np.empty((B, 1, OUT_H, OUT_W), dtype=np.float32)
    for c in range(N_CORES):
        b, half = divmod(c, 2)
        out[b, 0, half * OROWS : (half + 1) * OROWS] = res.results[c]["out"]
    return out



# revision 26
# speedup vs baseline: 2.6888x; 2.6888x over previous
"""Distributed kNN OOD-score kernel for 8 Trainium2 NeuronCores.

Problem: for each of 4*32*32 query vectors (D=768), find the 3 nearest
database vectors (N=20000, squared-L2), average the 3 distances, and
bilinearly upsample the resulting [4,32,32] map to [4,1,512,512].

Sharding: queries are data-parallel. Each core owns half of one batch
image (16 of 32 query rows = 512 queries); the database is replicated
and streamed through SBUF in fp8 (e4m3). The one halo row each core
needs for the 16x bilinear upsample is exchanged with its pair core via
a tiny AllGather, scheduled early (boundary query tile first, its last
two super-chunks processed ahead of the other tiles) so its ~15us
latency hides under the remaining compute stream.

Per-core device program (v3, fp8 DoubleRow + PE-side max-fold):
  - scores t[q,n] = q.x - ||x||^2/2 via TensorE fp8 e4m3 DoubleRow
    matmuls (2 K=128 chunks per pass, 0.5 cyc/col): 3 data passes + 1
    pass adding -||x||^2/2 as a 4-level fp8 split (scaled by 4), f32
    PSUM accumulate.
  - HALF the database columns are pre-folded in pairs on the host:
    for a pair (x0, x1) the device scores a=(x0+x1)/2 and b=(x0-x1)/2
    streams, ScalarE takes |v| of the b-scores, and TensorE adds it
    onto the a-scores with a bf16 identity matmul:
        u + |v| = max(t0, t1)   (exact)
    halving what VectorE has to scan for those columns. Column axis is
    slot-packed per 2000-col half-strip: [1000 direct | 500 a | 500 b].
  - VectorE max8 reads the direct banks and folded bank straight from
    PSUM (strided APs, skipping unused bank tails): 2500 elems per
    half-strip instead of 4000 -> PE and DVE balance at ~1.9us/half.
  - per query-tile: max8 over the 20 strip top-8s -> top-3, then
    mean distance = reduce_sum of sqrt((q^2 - 2t)/9).
  - pair AllGather of the boundary tile's 128 ood values (bf16).
  - 16x bilinear upsample = two small bf16 matmuls (interp weights are
    odd/32 fractions - exact in bf16; verified vs jax.image.resize).
"""

import sys

if "/opt/trn_rl_repo" not in sys.path:
    sys.path.insert(0, "/opt/trn_rl_repo")

import numpy as np
import ml_dtypes

import concourse.bass as bass
import concourse.bacc as bacc
import concourse.mybir as mybir
import concourse.tile as tile
from concourse import bass_utils

# Problem shape (hardcoded per contract).
B, D, H, W = 4, 768, 32, 32
N = 20000
K_NN = 3
OUT_H = OUT_W = 512
N_CORES = 8

SC = 4000            # db columns per super-chunk (8 PSUM banks * 500)
N_SC = N // SC       # 5
N_HS = 2 * N_SC      # 2000-col half-strips per query tile
QPC = 512            # queries scored per core (16 rows)
N_QT = QPC // 128    # 4
NKP = 3              # K pairs: 768 = 3 * (2*128)
OROWS = 256          # output rows per core
NCOL = 24            # ood columns entering the upsample (16 own + 2x4 gathered)
XS = 4.0             # scale folded into the xh stationary (fp8 range fit)
DEBUG = False        # adds intermediate-tensor outputs for debugging

F32 = mybir.dt.float32
BF16 = mybir.dt.bfloat16
FP8 = mybir.dt.float8e4
AX = mybir.AxisListType
AF = mybir.ActivationFunctionType
DR = mybir.MatmulPerfMode.DoubleRow

# local tile -> 4-row block of this core's half (block i = rows 4i..4i+3).
# Tile 0 is the block the PAIR core needs as its halo row: for the top
# half (rows 0-15) that's block 3 (row 15), for the bottom half (rows
# 16-31) block 0 (row 16).
TILE_BLOCKS = ([3, 0, 1, 2], [0, 1, 2, 3])

# Strip processing order: boundary query tile (qt 0) gets its last two
# super-chunks early so its AllGather launches ~15us before the stream
# ends.
STRIPS = (
    [(sc, qt) for sc in range(3) for qt in range(N_QT)]
    + [(3, 0), (4, 0)]
    + [(3, qt) for qt in range(1, N_QT)]
    + [(4, qt) for qt in range(1, N_QT)]
)


def _build_program():
    nc = bacc.Bacc(
        "TRN2", target_bir_lowering=False, debug=False, num_devices=N_CORES
    )
    q8d = nc.dram_tensor("q8", [128, NKP, 2, QPC], FP8, kind="ExternalInput").ap()
    db8d = nc.dram_tensor("db8", [128, NKP, 2, N], FP8, kind="ExternalInput").ap()
    xh4d = nc.dram_tensor("xh4", [2, 2, N], FP8, kind="ExternalInput").ap()
    q2d = nc.dram_tensor("q2", [128, N_QT], F32, kind="ExternalInput").ap()
    identd = nc.dram_tensor("ident", [128, 128], BF16, kind="ExternalInput").ap()
    artd = nc.dram_tensor("art", [NCOL, OROWS], BF16, kind="ExternalInput").ap()
    acd = nc.dram_tensor("ac", [W, OUT_W], BF16, kind="ExternalInput").ap()
    out = nc.dram_tensor("out", [OROWS, OUT_W], F32, kind="ExternalOutput").ap()
    if DEBUG:
        dbg_parts = nc.dram_tensor(
            "dbg_parts", [128, N_HS * 16], F32, kind="ExternalOutput"
        ).ap()
        dbg_oodht = nc.dram_tensor(
            "dbg_oodht", [W, NCOL], BF16, kind="ExternalOutput"
        ).ap()

    with tile.TileContext(nc) as tc:
        with (
            tc.tile_pool(name="static", bufs=1) as sp,
            tc.tile_pool(name="dbh", bufs=6) as dbhp,
            tc.tile_pool(name="db", bufs=9) as dbp,
            tc.tile_pool(name="absv", bufs=3) as avp,
            tc.tile_pool(name="small", bufs=4) as smp,
            tc.tile_pool(name="psum", bufs=2, space="PSUM") as pp,
            tc.tile_pool(name="dram", bufs=1, space="DRAM") as dp,
        ):
            # queries first (needed by the very first matmul), split per
            # K-pair so the first matmul only waits for kp 0
            q8 = sp.tile([128, NKP, 2, QPC], FP8)
            for kp in range(NKP):
                nc.sync.dma_start(q8[:, kp], q8d[:, kp])

            # super-chunk 0 as 2-col-half tiles for fast startup;
            # sc 1..4 as full [128, 2, 4000] tiles per K-pair.
            db0 = {}  # (kp, h) -> tile  (h: slots h*2000..h*2000+2000)
            for h in range(2):
                for kp in range(NKP):
                    t = dbhp.tile([128, 2, 2000], FP8, tag="dbh", name=f"db0_{kp}_{h}")
                    if h == 0 and kp == 0:
                        # split the very first db tile so matmuls start
                        # as early as possible
                        nc.sync.dma_start(
                            t[:, :, 0:1000], db8d[:, kp, :, 0:1000]
                        )
                        nc.sync.dma_start(
                            t[:, :, 1000:2000], db8d[:, kp, :, 1000:2000]
                        )
                    else:
                        nc.sync.dma_start(
                            t[:], db8d[:, kp, :, h * 2000 : (h + 1) * 2000]
                        )
                    db0[(kp, h)] = t
                    if h == 0 and kp == 0:
                        # small inputs the first strip needs, right after
                        # the first db tile so the pipeline starts early
                        xh4 = sp.tile([2, 2, N], FP8)
                        nc.sync.dma_start(xh4[:], xh4d[:])
                        q2_sb = sp.tile([128, N_QT], F32)
                        nc.sync.dma_start(q2_sb[:], q2d[:])
                        ident = sp.tile([128, 128], BF16)
                        nc.sync.dma_start(ident[:], identd[:])
            art_sb = sp.tile([NCOL, OROWS], BF16)
            nc.sync.dma_start(art_sb[:], artd[:])
            ac_sb = sp.tile([W, OUT_W], BF16)
            nc.sync.dma_start(ac_sb[:], acd[:])
            dbt = {}  # sc -> [kp] tiles
            for sc in range(1, N_SC):
                tiles = []
                for kp in range(NKP):
                    t = dbp.tile([128, 2, SC], FP8, tag="db", name=f"db{sc}_{kp}")
                    nc.sync.dma_start(
                        t[:], db8d[:, kp, :, sc * SC : (sc + 1) * SC]
                    )
                    tiles.append(t)
                dbt[sc] = tiles
            ones4 = sp.tile([2, 2, 128], FP8)
            nc.gpsimd.memset(ones4[:], XS)

            # per-query-tile top-8s: 2 per half-strip (direct, folded)
            parts = [
                sp.tile([128, N_HS * 16], F32, name=f"part{qt}")
                for qt in range(N_QT)
            ]
            oods = [
                sp.tile([128, 1], BF16, name=f"ood{qt}") for qt in range(N_QT)
            ]
            cc_in = dp.tile([128], BF16)
            cc_out = dp.tile([256], BF16)
            scratch_d = dp.tile([QPC], BF16)
            # ood_hT[c, j]: j 0..15 own rows (local order), 16..23 the two
            # gathered boundary blocks in rank order; filled incrementally
            # as each query tile finishes
            ood_hT = sp.tile([W, NCOL], BF16)

            def rhs(sc, kp, col, width):
                """db slots [col, col+width) of super-chunk sc, K-pair kp."""
                if sc == 0:
                    h, off = divmod(col, 2000)
                    return db0[(kp, h)][:, :, off : off + width]
                return dbt[sc][kp][:, :, col : col + width]

            def qt_epilogue(qt):
                f8 = smp.tile([128, 8], F32, tag="f8", name=f"f8_{qt}")
                nc.vector.max(f8[:], parts[qt][:])
                # dist_j/3 = sqrt((q2 - 2 t_j) / 9); host passes q2/9
                d3 = smp.tile([128, K_NN], F32, tag="d3", name=f"d3_{qt}")
                nc.scalar.activation(
                    d3[:],
                    f8[:, 0:K_NN],
                    AF.Sqrt,
                    bias=q2_sb[:, qt : qt + 1],
                    scale=-2.0 / 9.0,
                )
                with nc.allow_low_precision(
                    reason="3-element sum rounded to bf16 for the ood "
                    "exchange; ~0.2% on a 2% tolerance"
                ):
                    nc.vector.reduce_sum(oods[qt][:], d3[:], axis=AX.X)
                if qt == 0:
                    # boundary block: gather it across the pair ASAP so
                    # the ~15us collective hides under remaining work
                    nc.sync.dma_start(cc_in[:], oods[0][:])
                    nc.gpsimd.collective_compute(
                        "AllGather",
                        mybir.AluOpType.bypass,
                        replica_groups=[[0, 1], [2, 3], [4, 5], [6, 7]],
                        ins=[cc_in.opt()],
                        outs=[cc_out.opt()],
                    )
                    nc.sync.dma_start(
                        ood_hT[:, 16:NCOL],
                        cc_out.rearrange("(b r c) -> c (b r)", b=2, c=W),
                    )
                # own block into the upsample operand via a DRAM round-trip
                # (the DMA engine transposes on the DRAM side); per-qt so
                # only the last tile's hop is on the critical path
                nc.sync.dma_start(
                    scratch_d.rearrange("(q p) -> p q", p=128)[:, qt : qt + 1],
                    oods[qt][:],
                )
                nc.sync.dma_start(
                    ood_hT[:, qt * 4 : (qt + 1) * 4],
                    scratch_d.rearrange("(q b c) -> c (q b)", q=N_QT, c=W)[
                        :, qt * 4 : (qt + 1) * 4
                    ],
                )

            # deferred per-half-strip work: (u_tile, absv_tile, part_ap)
            pending = []

            def drain_one():
                u, absv, part_ap = pending.pop(0)
                # u += |v| closes the folded accumulation group (exact
                # max: u + |v| = max(t0, t1))
                nc.tensor.matmul(
                    u[:, 0, 0:500], ident[:], absv[:],
                    start=False, stop=True,
                )
                nc.vector.max(part_ap, u[:, 0, 0:500])

            for si, (sc, qt) in enumerate(STRIPS):
                lhsT = [
                    q8[:, kp, :, qt * 128 : (qt + 1) * 128] for kp in range(NKP)
                ]
                for hf in range(2):
                    s0 = hf * 2000          # in-chunk slot base
                    g0 = sc * SC + s0       # global slot base
                    pslc = parts[qt][:, (sc * 2 + hf) * 16 :]

                    def emit_direct():
                        # direct banks: slots [s0, s0+1000)
                        dt_ps = pp.tile([128, 2, 512], F32, tag="dt", name="dt")
                        for kp in range(NKP):
                            for b in range(2):
                                nc.tensor.matmul(
                                    dt_ps[:, b, 0:500],
                                    lhsT[kp],
                                    rhs(sc, kp, s0 + b * 500, 500),
                                    start=(kp == 0),
                                    stop=False,
                                    perf_mode=DR,
                                )
                        for b in range(2):
                            nc.tensor.matmul(
                                dt_ps[:, b, 0:500],
                                ones4[:],
                                xh4[:, :, g0 + b * 500 : g0 + (b + 1) * 500],
                                start=False,
                                stop=True,
                                perf_mode=DR,
                            )
                        nc.vector.max(pslc[:, 0:8], dt_ps[:, :, 0:500])

                    def emit_folded():
                        # folded columns: a-slots [s0+1000, s0+1500),
                        # b-slots [s0+1500, s0+2000)
                        u_ps = pp.tile([128, 1, 512], F32, tag="u", name="u")
                        v_ps = pp.tile([128, 1, 512], F32, tag="v", name="v")
                        for kp in range(NKP):
                            nc.tensor.matmul(
                                u_ps[:, 0, 0:500], lhsT[kp],
                                rhs(sc, kp, s0 + 1000, 500),
                                start=(kp == 0), stop=False, perf_mode=DR,
                            )
                            nc.tensor.matmul(
                                v_ps[:, 0, 0:500], lhsT[kp],
                                rhs(sc, kp, s0 + 1500, 500),
                                start=(kp == 0), stop=False, perf_mode=DR,
                            )
                        nc.tensor.matmul(
                            u_ps[:, 0, 0:500], ones4[:],
                            xh4[:, :, g0 + 1000 : g0 + 1500],
                            start=False, stop=False, perf_mode=DR,
                        )
                        nc.tensor.matmul(
                            v_ps[:, 0, 0:500], ones4[:],
                            xh4[:, :, g0 + 1500 : g0 + 2000],
                            start=False, stop=True, perf_mode=DR,
                        )
                        # ScalarE: |v| -> SBUF bf16
                        absv = avp.tile([128, 500], BF16, tag="absv", name="absv")
                        nc.scalar.activation(absv[:], v_ps[:, 0, 0:500], AF.Abs)
                        pending.append((u_ps, absv, pslc[:, 8:16]))

                    if si < len(STRIPS) - 2:
                        emit_direct()
                        emit_folded()
                        # drain the PREVIOUS half-strip's id-add + max8 here:
                        # by now its |v| activation has finished, so neither
                        # PE nor DVE stalls on the ScalarE chain
                        while len(pending) > 1:
                            drain_one()
                    else:
                        # tail strips: fold work first so its Abs/id/max8
                        # chain overlaps the direct matmuls, then drain
                        emit_folded()
                        emit_direct()
                        while pending:
                            drain_one()

                is_qt_last = (sc, qt) in ((4, 0), (4, 1), (4, 2), (4, 3))
                if is_qt_last:
                    while pending:
                        drain_one()
                    qt_epilogue(qt)

            if DEBUG:
                nc.sync.dma_start(dbg_parts[:], parts[0][:])
                nc.sync.dma_start(dbg_oodht[:], ood_hT[:])

            # P1[j, ow] = sum_c ood_hT[c, j] * A_c[c, ow]
            p1 = pp.tile([NCOL, OUT_W], F32, tag="dt", name="p1")
            nc.tensor.matmul(p1[:], ood_hT[:], ac_sb[:], start=True, stop=True)
            p1_sb = sp.tile([NCOL, OUT_W], BF16)
            nc.scalar.activation(p1_sb[:], p1[:], AF.Copy)
            # out[oi, ow] = sum_j art[j, oi] * P1[j, ow]
            for m in range(2):
                p2 = pp.tile([128, OUT_W], F32, tag="dt", name=f"p2_{m}")
                nc.tensor.matmul(
                    p2[:],
                    art_sb[:, m * 128 : (m + 1) * 128],
                    p1_sb[:],
                    start=True,
                    stop=True,
                )
                o_sb = smp.tile([128, OUT_W], F32, tag="osb", name=f"osb{m}")
                nc.scalar.activation(o_sb[:], p2[:], AF.Copy)
                nc.sync.dma_start(out[m * 128 : (m + 1) * 128, :], o_sb[:])

    nc.compile()
    return nc


def _bilinear_matrix(out_size: int, in_size: int) -> np.ndarray:
    """Half-pixel (align_corners=False) bilinear interpolation matrix
    [out_size, in_size]; edge-clamped, equivalent to jax.image.resize
    'bilinear' for integer upsampling."""
    A = np.zeros((out_size, in_size), dtype=np.float64)
    scale = in_size / out_size
    for i in range(out_size):
        s = (i + 0.5) * scale - 0.5
        j0 = int(np.floor(s))
        w = s - j0
        A[i, min(max(j0, 0), in_size - 1)] += 1.0 - w
        A[i, min(max(j0 + 1, 0), in_size - 1)] += w
    return A.astype(np.float32)


_NC_CACHE = None


def _get_nc():
    global _NC_CACHE
    if _NC_CACHE is None:
        _NC_CACHE = _build_program()
    return _NC_CACHE


def _slot_pack(database: np.ndarray):
    """Transform db columns into the slot-packed layout: per 2000-col
    half-strip [1000 direct | 500 a=(x0+x1)/2 | 500 b=(x0-x1)/2], with
    the matching -||x||^2/2-style xh terms per slot."""
    x2h = 0.5 * np.einsum("nd,nd->n", database, database)  # ||x||^2/2
    dbX = np.empty_like(database)                          # [N, D] slot-major
    xhX = np.empty(N, dtype=np.float32)
    for hs in range(N // 2000):
        base = hs * 2000
        d = slice(base, base + 1000)
        dbX[d] = database[d]
        xhX[d] = -x2h[d]
        p0 = database[base + 1000 : base + 2000 : 2]
        p1 = database[base + 1001 : base + 2000 : 2]
        h0 = x2h[base + 1000 : base + 2000 : 2]
        h1 = x2h[base + 1001 : base + 2000 : 2]
        dbX[base + 1000 : base + 1500] = 0.5 * (p0 + p1)
        dbX[base + 1500 : base + 2000] = 0.5 * (p0 - p1)
        xhX[base + 1000 : base + 1500] = -0.5 * (h0 + h1)
        xhX[base + 1500 : base + 2000] = -0.5 * (h0 - h1)
    return dbX, xhX


def make_in_maps(embeddings: np.ndarray, database: np.ndarray):
    embeddings = np.asarray(embeddings, dtype=np.float32)
    database = np.asarray(database, dtype=np.float32)

    dbX, xhX = _slot_pack(database)

    # db in fp8, contraction-pair layout: db8[p, kp, i, n] = dbX[n, kp*256+i*128+p]
    db8 = np.ascontiguousarray(
        dbX.T.reshape(NKP, 2, 128, N).transpose(2, 0, 1, 3)
    ).astype(ml_dtypes.float8_e4m3)

    # per-slot xh term, scaled by 1/XS, as a 4-level fp8 split
    levels = []
    r = (xhX / XS).astype(np.float32)
    for _ in range(4):
        h = r.astype(ml_dtypes.float8_e4m3)
        levels.append(h)
        r = r - h.astype(np.float32)
    # xh4[p, i, :]: 4 levels at (p0,i0),(p1,i0),(p0,i1),(p1,i1)
    xh4 = np.stack(
        [np.stack([levels[0], levels[2]]), np.stack([levels[1], levels[3]])]
    )  # [2, 2, N]

    q_all = embeddings.transpose(0, 2, 3, 1).reshape(B, H * W, D)
    Ac = _bilinear_matrix(OUT_W, W)                      # [512, 32]
    Ar = _bilinear_matrix(OUT_H, H)                      # [512, 32]
    ident = np.eye(128, dtype=np.float32).astype(ml_dtypes.bfloat16)
    # the two gathered blocks in cc_out rank order: pair-core tile 0 rows
    cc_rows = [12, 13, 14, 15, 16, 17, 18, 19]

    in_maps = []
    for c in range(N_CORES):
        b, half = divmod(c, 2)
        blocks = TILE_BLOCKS[half]
        own_rows = [16 * half + 4 * blk + r for blk in blocks for r in range(4)]

        # queries in local-tile order
        q = np.concatenate(
            [
                q_all[b, (16 * half + 4 * blk) * W : (16 * half + 4 * blk + 4) * W]
                for blk in blocks
            ]
        )                                                # [512, 768]
        q8 = np.ascontiguousarray(
            q.T.reshape(NKP, 2, 128, QPC).transpose(2, 0, 1, 3)
        ).astype(ml_dtypes.float8_e4m3)                  # [128, 3, 2, 512]
        q2 = np.einsum("qd,qd->q", q, q) / 9.0
        q2 = np.ascontiguousarray(q2.reshape(N_QT, 128).T.astype(np.float32))

        # interpolation rows matching ood_hT's column order
        Arh = Ar[half * OROWS : (half + 1) * OROWS]      # [256, 32]
        art = np.zeros((NCOL, OROWS), dtype=np.float32)
        for j, row in enumerate(own_rows):
            art[j] = Arh[:, row]
        for j, row in enumerate(cc_rows):
            if row not in own_rows:
                art[16 + j] = Arh[:, row]
        in_maps.append(
            {
                "db8": db8,
                "xh4": xh4,
                "q8": q8,
                "q2": q2,
                "ident": ident,
                "art": art.astype(ml_dtypes.bfloat16),
                "ac": np.ascontiguousarray(Ac.T).astype(ml_dtypes.bfloat16),
            }
        )
    return in_maps


def run_device(in_maps, **kwargs):
    nc = _get_nc()
    return bass_utils.run_bass_kernel_spmd(
        nc, in_maps, core_ids=list(range(N_CORES)), **kwargs
    )


def kernel(embeddings, database, k, out_h, out_w):
    assert int(k) == K_NN and int(out_h) == OUT_H and int(out_w) == OUT_W
    in_maps = make_in_maps(np.asarray(embeddings), np.asarray(database))
    res = run_device(in_maps)
    out = np.empty((B, 1, OUT_H, OUT_W), dtype=np.float32)
    for c in range(N_CORES):
        b, half = divmod(c, 2)
        out[b, 0, half * OROWS : (half + 1) * OROWS] = res.results[c]["out"]
    return out
